# revision 31
# baseline (speedup 1.0000x reference)
"""Multi-head attention (B=8, S=1024, D=1024, H=16) on 8 trn2 NeuronCores.

Strategy: batch-parallel (1 batch per core), zero collectives.
Per core, everything is computed in "transposed" layouts so that no on-device
transposes are needed:
  - host passes x^T-prepped inputs (bf16), so projections produce q^T/k^T
    [e, s] (e on partitions) and v [t, e] directly;
  - scores are computed transposed ([t, s]) with 64-partition contractions
    (per-head kT slices live at their natural partition base, no zero pad);
  - softmax denominator comes from an extra ones-column appended to v
    (row sums of exp via the same matmul);
  - attention output lands as cat^T [e, s], output projection produces
    out^T [f, s] with bo as per-partition bias; host transposes back.
All matmuls run in bf16 (FWL + pipelined LDWEIGHTS) with fp32 PSUM accumulate.
Schedule notes:
  - dummy warm-up matmuls run during the startup DMA window so the PE reaches
    full p-state before real work arrives;
  - the attention phase is ACT(exp)-throttled, so the next pair's k/q
    projections are split into pieces and interleaved into the attention loop
    as PE filler (avoids cold-p-state restarts after ACT waits);
  - the second matmul of each same-stationary pair (scores/AV s-halves) skips
    its LDWEIGHTS via InstMatmult.ldweights=False;
  - the output projection is split: et=0..3 partials run as pair-5/6 fillers,
    et=4..6 as pair-7 fillers (folding in the first partial), and only the
    et=7 term + bias remain as the tail.
"""

import sys

if "/opt/trn_rl_repo" not in sys.path:
    sys.path.insert(0, "/opt/trn_rl_repo")

from collections import deque

import ml_dtypes
import numpy as np

B, S, D, H = 8, 1024, 1024, 16
Dh = D // H  # 64
P = 128
NT = 8  # number of 128-row tiles in 1024
SH = 512  # s-half

BF16 = ml_dtypes.bfloat16
LDW_SKIP = True

_CACHE = {}


def _prep_x(x):
    # x [S, D] -> [2, 128, 4096]; out[hf, p, k*512 + s'] = x[hf*512+s', k*128+p]
    return np.ascontiguousarray(
        x.reshape(2, SH, NT, P).transpose(0, 3, 2, 1).astype(BF16)
    ).reshape(2, P, NT * SH)


def _prep_w(Wcat):
    # W [out 1024, in 1024] -> [8, 128, 1024]; out[ot, p, k*128+oc] = W[ot*128+oc, k*128+p]
    return np.ascontiguousarray(
        Wcat.reshape(NT, P, NT, P).transpose(0, 3, 2, 1).astype(BF16)
    ).reshape(NT, P, NT * P)


def _prep_wv(Wvcat):
    # rhs layout [8, 128, 1024]; out[k, p, e] = Wv_cat[e, k*128+p]
    return np.ascontiguousarray(Wvcat.T.reshape(NT, P, D).astype(BF16))


def _prep_bias(b):
    # [1024] -> [128, 8]; out[p, i] = b[i*128+p]
    return np.ascontiguousarray(b.reshape(NT, P).T)


def _build():
    import concourse.mybir as mybir
    import concourse.tile as tile
    from concourse import bacc

    dt = mybir.dt
    f32 = dt.float32
    bf16 = dt.bfloat16
    AF = mybir.ActivationFunctionType
    ADD = mybir.AluOpType.add

    nc = bacc.Bacc(None, target_bir_lowering=False)

    with tile.TileContext(nc) as tc:
        with (
            tc.tile_pool(name="dram", bufs=1, space="DRAM") as dram,
            tc.tile_pool(name="consts", bufs=1) as consts,
            tc.tile_pool(name="xh_p", bufs=1) as xh_p,
            tc.tile_pool(name="wst_p", bufs=3) as wst_p,
            tc.tile_pool(name="wv_p", bufs=1) as wv_p,
            tc.tile_pool(name="vaug_p", bufs=1) as vaug_p,
            tc.tile_pool(name="cat_p", bufs=1) as cat_p,
            tc.tile_pool(name="qp_p", bufs=2) as qp_p,
            tc.tile_pool(name="kp_p", bufs=2) as kp_p,
            tc.tile_pool(name="ex_p", bufs=8) as ex_p,
            tc.tile_pool(name="rc_p", bufs=2) as rc_p,
            tc.tile_pool(name="bc_p", bufs=2) as bc_p,
            tc.tile_pool(name="tm_p", bufs=2) as tm_p,
            tc.tile_pool(name="st_p", bufs=6) as st_p,
            tc.tile_pool(name="ps", bufs=2, space="PSUM") as ps_p,
        ):
            # ---- DRAM I/O ----
            xq = dram.tile([2, P, NT * SH], bf16, kind="ExternalInput", name="xq", uniquify=False)
            xk = dram.tile([2, P, NT * SH], bf16, kind="ExternalInput", name="xk", uniquify=False)
            xv = dram.tile([2, P, NT * SH], bf16, kind="ExternalInput", name="xv", uniquify=False)
            wq = dram.tile([NT, P, D], bf16, kind="ExternalInput", name="wq", uniquify=False)
            wk = dram.tile([NT, P, D], bf16, kind="ExternalInput", name="wk", uniquify=False)
            wv = dram.tile([NT, P, D], bf16, kind="ExternalInput", name="wv", uniquify=False)
            wo = dram.tile([NT, P, D], bf16, kind="ExternalInput", name="wo", uniquify=False)
            bqd = dram.tile([P, NT], f32, kind="ExternalInput", name="bqd", uniquify=False)
            bkd = dram.tile([P, NT], f32, kind="ExternalInput", name="bkd", uniquify=False)
            bod = dram.tile([P, NT], f32, kind="ExternalInput", name="bod", uniquify=False)
            outT = dram.tile([NT, P, S], f32, kind="ExternalOutput", name="outT", uniquify=False)
            rcd = dram.tile([NT, 2, S], f32, name="rcd")
            rcd2 = dram.tile([NT, 2, S], f32, name="rcd2")

            # ---- PE warm-up: keep the array busy through the DMA window so
            # the p-state ramps to full clock before the first projection.
            warm = consts.tile([P, SH], bf16, name="warm")
            nc.vector.memset(warm[:], 0.0)
            wps = ps_p.tile([P, SH], f32, name="wps", tag="pp", bufs=2)
            for _ in range(24):
                nc.tensor.matmul(wps[:], warm[:, 0:P], warm[:], start=True, stop=True)

            # ---- startup DMAs, ordered by first use on the PE ----
            xhk, xhq, xhv = [], [], []
            for name, src, dst in (("xk", xk, xhk), ("xq", xq, xhq), ("xv", xv, xhv)):
                for hf in range(2):
                    t = xh_p.tile([P, NT * SH], bf16, name=f"{name}{hf}", tag=f"{name}{hf}")
                    dst.append(t)

            bq_sb = consts.tile([P, NT], f32, name="bq_sb")
            bk_sb = consts.tile([P, NT], f32, name="bk_sb")
            bo_sb = consts.tile([P, NT], f32, name="bo_sb")

            wkt0 = wst_p.tile([P, D], bf16, name="wkt", tag="w")
            nc.sync.dma_start(wkt0[:], wk[0])
            nc.sync.dma_start(xhk[0][:], xk[0])
            nc.sync.dma_start(bk_sb[:], bkd[:])
            wqt0 = wst_p.tile([P, D], bf16, name="wqt", tag="w")
            nc.sync.dma_start(wqt0[:], wq[0])
            nc.sync.dma_start(xhq[0][:], xq[0])
            nc.sync.dma_start(bq_sb[:], bqd[:])
            nc.sync.dma_start(xhk[1][:], xk[1])
            nc.sync.dma_start(xhq[1][:], xq[1])
            nc.sync.dma_start(bo_sb[:], bod[:])
            nc.sync.dma_start(xhv[0][:], xv[0])
            nc.sync.dma_start(xhv[1][:], xv[1])

            v_aug = vaug_p.tile([P, NT, H, Dh + 1], bf16, name="v_aug")
            nc.vector.memset(v_aug[:, :, :, Dh], 1.0)
            wv_sb = wv_p.tile([P, NT * D], bf16, name="wv_sb")
            for k in range(NT):
                nc.sync.dma_start(wv_sb[:, k * D : (k + 1) * D], wv[k])
            wo_sb = wv_p.tile([P, NT * D], bf16, name="wo_sb")

            catT = cat_p.tile([P, NT, S], bf16, name="catT")
            # o-proj partial sums, staged in bf16: sta1 = et 0..3, sta2 = + et 4..6
            sta1 = cat_p.tile([P, 2, NT, SH], bf16, name="sta1")
            sta2 = cat_p.tile([P, 2, NT, SH], bf16, name="sta2")

            # ---------- chain builders: lists of single-op closures ----------
            def kq_chain(w, xh, hf, bias, out):
                state = {}

                def mm(k):
                    def f():
                        if k == 0:
                            state["ps"] = ps_p.tile(
                                [P, SH], f32, name="pp", tag="pp", bufs=2
                            )
                        nc.tensor.matmul(
                            state["ps"][:],
                            w[:, k * P : (k + 1) * P],
                            xh[:, k * SH : (k + 1) * SH],
                            start=(k == 0),
                            stop=(k == NT - 1),
                        )

                    return f

                def fin():
                    nc.vector.tensor_scalar_add(
                        out[:, hf * SH : (hf + 1) * SH], state["ps"][:], bias
                    )

                return [mm(k) for k in range(NT)] + [fin]

            def vp_chain(eh, tt):
                # v [t-block tt, heads eh*8..eh*8+7] via x^T stationary, WvT moving
                hf, tl = divmod(tt, 4)
                state = {}

                def mm(k):
                    def f():
                        if k == 0:
                            state["ps"] = ps_p.tile(
                                [P, SH], f32, name="pv", tag="pp", bufs=2
                            )
                        nc.tensor.matmul(
                            state["ps"][:],
                            xhv[hf][:, k * SH + tl * P : k * SH + (tl + 1) * P],
                            wv_sb[:, k * D + eh * SH : k * D + (eh + 1) * SH],
                            start=(k == 0),
                            stop=(k == NT - 1),
                        )

                    return f

                def fin():
                    nc.vector.tensor_copy(
                        v_aug[:, tt, eh * 8 : (eh + 1) * 8, 0:Dh],
                        state["ps"][:].rearrange("p (g c) -> p g c", c=Dh),
                    )

                return [mm(k) for k in range(NT)] + [fin]

            def oproj_chain(ft, sh, part):
                # output projection partials: part 1 = et 0..2 -> sta1;
                # part 2 = et 3..6, fin adds sta1 -> sta2 (the longer part-2
                # chain amortizes the PSUM-ring fin latency in pair 7)
                ets = range(0, 3) if part == 1 else range(3, 7)
                state = {}

                def mm(et):
                    def f():
                        if et == ets[0]:
                            state["ps"] = ps_p.tile(
                                [P, SH], f32, name="poa", tag="pp", bufs=2
                            )
                        nc.tensor.matmul(
                            state["ps"][:],
                            wo_sb[:, ft * D + et * P : ft * D + (et + 1) * P],
                            catT[:, et, sh * SH : (sh + 1) * SH],
                            start=(et == ets[0]),
                            stop=(et == ets[-1]),
                        )

                    return f

                def fin():
                    if part == 1:
                        nc.vector.tensor_copy(sta1[:, sh, ft, :], state["ps"][:])
                    else:
                        nc.vector.tensor_add(
                            sta2[:, sh, ft, :], state["ps"][:], sta1[:, sh, ft, :]
                        )

                return [mm(et) for et in ets] + [fin]

            # ---- prologue: k/q projection for pair 0 (nothing to overlap);
            # hf0 chains first (their x halves arrive first), then hf1, then
            # the first v-proj chain so av(0,0,0) finds v_aug tile 0 ready.
            kz = kp_p.tile([P, S], bf16, name="kz", tag="kz")
            qp = qp_p.tile([P, S], bf16, name="qp", tag="qp")
            for hf in range(2):
                for op in kq_chain(wkt0, xhk[hf], hf, bk_sb[:, 0:1], kz):
                    op()
                for op in kq_chain(wqt0, xhq[hf], hf, bq_sb[:, 0:1], qp):
                    op()
            for op in vp_chain(0, 0):
                op()

            # eh1 v-proj chains spread over pairs 1-3 (first needed at pair 4)
            vjit_eh1 = {1: [0, 1, 2], 2: [3, 4, 5], 3: [6, 7]}

            # ---- per head-pair: attention with interleaved filler pieces ----
            for pr in range(NT):
                fq = deque()
                if pr == 2:
                    # o-proj weights, needed from pair 5 on; mid-kernel the DMA
                    # queues are quiet
                    for k in range(NT):
                        nc.sync.dma_start(wo_sb[:, k * D : (k + 1) * D], wo[k])
                if pr < NT - 1:
                    wkt = wst_p.tile([P, D], bf16, name="wkt", tag="w")
                    nc.sync.dma_start(wkt[:], wk[pr + 1])
                    wqt = wst_p.tile([P, D], bf16, name="wqt", tag="w")
                    nc.sync.dma_start(wqt[:], wq[pr + 1])
                    kz_n = kp_p.tile([P, S], bf16, name="kz", tag="kz")
                    qp_n = qp_p.tile([P, S], bf16, name="qp", tag="qp")
                    for hf in range(2):
                        fq.extend(kq_chain(wkt, xhk[hf], hf, bk_sb[:, pr + 1 : pr + 2], kz_n))
                    for hf in range(2):
                        fq.extend(kq_chain(wqt, xhq[hf], hf, bq_sb[:, pr + 1 : pr + 2], qp_n))
                for tt in vjit_eh1.get(pr, ()):
                    fq.extend(vp_chain(1, tt))
                a1 = [(ft, sh) for ft in range(NT) for sh in range(2)]
                if pr == 4:
                    for ft, sh in a1[0:5]:
                        fq.extend(oproj_chain(ft, sh, 1))
                elif pr == 5:
                    for ft, sh in a1[5:10]:
                        fq.extend(oproj_chain(ft, sh, 1))
                elif pr == 6:
                    for ft, sh in a1[10:16]:
                        fq.extend(oproj_chain(ft, sh, 1))
                elif pr == 7:
                    for ft, sh in a1:
                        fq.extend(oproj_chain(ft, sh, 2))

                slots = [15 if pr == 0 else 22]

                def pop_fill(last=False):
                    if not fq:
                        return
                    n = len(fq) if last else -(-len(fq) // max(slots[0], 1))
                    for _ in range(min(n, len(fq))):
                        fq.popleft()()
                    slots[0] -= 1

                ajs = []
                for j in range(2):
                    h = 2 * pr + j
                    e0, e1 = j * Dh, (j + 1) * Dh
                    av = ps_p.tile([Dh + 1, S], f32, name="av", tag="av", bufs=1)
                    exs = {}
                    # pair-0 head-0 runs while the xv/Wv DMAs are still
                    # streaming: defer its av matmuls further so the JIT
                    # v-proj inputs have arrived by the time they issue
                    lag = 5 if (pr == 0 and j == 0) else 3

                    def av_pair(tt, start, stop):
                        for sh in range(2):
                            r = nc.tensor.matmul(
                                av[:, sh * SH : (sh + 1) * SH],
                                v_aug[:, tt, h, :],
                                exs[tt][:, sh * SH : (sh + 1) * SH],
                                start=start,
                                stop=stop,
                            )
                            if sh == 1 and LDW_SKIP:
                                r.ins.ldweights = False

                    # super-steps of 2 score tiles: ACT always has 2 queued
                    # inputs, so PE/ACT phase jitter is absorbed by the queue
                    # instead of stalling the av matmuls
                    for tt2 in range(0, NT, 2):
                        for tt in (tt2, tt2 + 1):
                            sc = ps_p.tile([P, S], f32, name="sc", tag="sc", bufs=2)
                            for sh in range(2):
                                r = nc.tensor.matmul(
                                    sc[:, sh * SH : (sh + 1) * SH],
                                    kz[e0:e1, tt * P : (tt + 1) * P],
                                    qp[e0:e1, sh * SH : (sh + 1) * SH],
                                )
                                if sh == 1 and LDW_SKIP:
                                    r.ins.ldweights = False
                            ex = ex_p.tile([P, S], bf16, name="ex", tag="ex")
                            nc.scalar.activation(ex[:], sc[:], AF.Exp, scale=0.125)
                            exs[tt] = ex
                        pop_fill()
                        if pr == 0 and j == 0:
                            # JIT v-proj for heads 0-7, one tile ahead of av
                            # (tile 0 was produced in the prologue). Emitted
                            # after the filler pop: kq work has resident
                            # inputs, so the PE stays busy while the xv/Wv
                            # DMAs are still streaming in.
                            for tt in (tt2, tt2 + 1):
                                if tt + 1 < NT:
                                    for op in vp_chain(0, tt + 1):
                                        op()
                        for tt in (tt2, tt2 + 1):
                            if tt >= lag:
                                av_pair(tt - lag, tt == lag, False)
                        if not (pr == 0 and j == 0):
                            pop_fill()
                    for tt in range(NT - lag, NT):
                        pop_fill()
                        av_pair(tt, False, tt == NT - 1)
                    # evacuate promptly so the single av slot frees for head j+1
                    aj = tm_p.tile([Dh + 1, S], f32, name="aj", tag="aj")
                    nc.vector.tensor_copy(aj[:], av[:])
                    ajs.append(aj)
                    nc.sync.dma_start(rcd[pr, j : j + 1, :], aj[Dh : Dh + 1, :])
                pop_fill(last=True)
                if pr < NT - 1:
                    kz, qp = kz_n, qp_n
                # softmax denominators (after both heads so the DMA roundtrip
                # latency overlaps the next pair's attention instead of
                # blocking this pair's in-order DVE stream); lane p holds
                # s in [8p, 8p+8)
                rc2 = rc_p.tile([P, 2, NT], f32, name="rc2", tag="rc")
                nc.sync.dma_start(rc2[:], rcd[pr].rearrange("j (p g) -> p j g", g=NT))
                rc3 = rc_p.tile([P, 2, NT], f32, name="rc3", tag="rc")
                nc.vector.reciprocal(rc3[:], rc2[:])
                nc.sync.dma_start(
                    rcd2[pr].rearrange("j (p g) -> p j g", g=NT), rc3[:]
                )
                for j in range(2):
                    bc = bc_p.tile([Dh, S], f32, name="bc", tag="bc")
                    nc.sync.dma_start(
                        bc[:], rcd2[pr, j : j + 1, :].broadcast_to([Dh, S])
                    )
                    # pairs 0-5: normalize on the idle GPSIMD engine so the
                    # DVE stream never blocks on the bc DMA roundtrip; the
                    # catT consumer is pairs away. Pairs 6-7 stay on DVE
                    # (their catT gates the o-proj tail).
                    eng = nc.gpsimd if pr < 6 else nc.vector
                    eng.tensor_mul(
                        catT[j * Dh : (j + 1) * Dh, pr, :], ajs[j][0:Dh, :], bc[:]
                    )

            # ---- output projection tail: last contraction block + bias + sta2
            for ft in range(NT):
                pss = []
                for sh in range(2):
                    # alternate PSUM tags (attention's sc banks are free now)
                    # for an effective ring of 4 — the et7 matmuls never wait
                    # on the DVE adds draining
                    ps = ps_p.tile(
                        [P, SH], f32, name="pob",
                        tag="pp" if sh == 0 else "sc", bufs=2,
                    )
                    r = nc.tensor.matmul(
                        ps[:],
                        wo_sb[:, ft * D + (NT - 1) * P : ft * D + NT * P],
                        catT[:, NT - 1, sh * SH : (sh + 1) * SH],
                        start=True,
                        stop=True,
                    )
                    if sh == 1 and LDW_SKIP:
                        r.ins.ldweights = False
                    pss.append(ps)
                for sh in range(2):
                    st = st_p.tile([P, SH], f32, name="so", tag="st")
                    nc.vector.scalar_tensor_tensor(
                        st[:], pss[sh][:], bo_sb[:, ft : ft + 1],
                        sta2[:, sh, ft, :], ADD, ADD,
                    )
                    nc.sync.dma_start(outT[ft][:, sh * SH : (sh + 1) * SH], st[:])

    nc.compile()
    return nc


def kernel(query, key, value, mask, Wq, bq, Wk, bk, Wv, bv, Wo, bo):
    from concourse.bass_utils import run_bass_kernel_spmd

    if "nc" not in _CACHE:
        _CACHE["nc"] = _build()
    nc = _CACHE["nc"]

    query = np.asarray(query, np.float32)
    key = np.asarray(key, np.float32)
    value = np.asarray(value, np.float32)
    Wq_c = np.asarray(Wq, np.float32).reshape(D, D)
    Wk_c = np.asarray(Wk, np.float32).reshape(D, D)
    Wv_c = np.asarray(Wv, np.float32).reshape(D, D)
    Wo_c = np.asarray(Wo, np.float32)
    bq_c = np.asarray(bq, np.float32).reshape(D)
    bk_c = np.asarray(bk, np.float32).reshape(D)
    bv_c = np.asarray(bv, np.float32).reshape(D)
    bo_c = np.asarray(bo, np.float32)

    shared = {
        "wq": _prep_w(Wq_c),
        "wk": _prep_w(Wk_c),
        "wv": _prep_wv(Wv_c),
        "wo": _prep_w(Wo_c),
        "bqd": _prep_bias(bq_c),
        "bkd": _prep_bias(bk_c),
        # attn rows sum to 1, so  attn @ (v + bv) = attn @ v + bv, and bv then
        # flows through the output projection as an extra bias Wo @ bv.
        "bod": _prep_bias(bo_c + Wo_c @ bv_c),
    }
    in_maps = []
    for b in range(B):
        m = dict(shared)
        m["xq"] = _prep_x(query[b])
        m["xk"] = _prep_x(key[b])
        m["xv"] = _prep_x(value[b])
        in_maps.append(m)

    res = run_bass_kernel_spmd(nc, in_maps, core_ids=list(range(B)))
    out = np.empty((B, S, D), np.float32)
    for b in range(B):
        out[b] = res.results[b]["outT"].reshape(D, S).T
    return out
